# revision 36
# baseline (speedup 1.0000x reference)
"""DeepSeekMoE forward on 8 TRN2 cores — gathered expert-parallel version.

Sharding as kernel.py (routed expert c -> core c, shared experts 8-way
H-sliced, router replicated). The routed FFN runs only on the ~N*topk/E
tokens routed to this core's expert:

  - compaction: top-2 mask -> per-tile counts + prefix sums (triangular
    fp32 matmuls, exact for small ints) -> compact slot per selected
    token (unselected -> out-of-range slot `cap`, which never matches)
  - slot tables WITHOUT indirect scatters (they are descriptor-rate
    bound): one-hot permutation tiles P[t,s] = (pos[t]==s) built by DVE
    compares, then tiny fp32 matmuls P.T @ [tile_id, part_id, gate]
    produce slot-ordered (token id, gate) tables in SBUF
  - indirect-DMA row gather of x for the selected tokens, PE-transpose
    into xgT [D, slot]
  - routed FFN (fp32r) on `cap` slots, gated by gathered gate (empty
    slots have gate 0); compact output rows [cap, D]
  - shared experts run densely over all tokens (emitted FIRST so the
    scheduler overlaps them with the whole compaction pipeline)
  - host: out = x + sum_c shared_c; out[idx_c[:cnt_c]] += routed_c[:cnt_c]
"""

import sys
from contextlib import ExitStack

if "/opt/trn_rl_repo" not in sys.path:
    sys.path.insert(0, "/opt/trn_rl_repo")

import numpy as np

import concourse.bass as bass
import concourse.mybir as mybir
import concourse.tile as tile
from concourse import bacc
from concourse.bass import IndirectOffsetOnAxis
from concourse.bass_utils import run_bass_kernel_spmd

F32 = mybir.dt.float32
F32R = mybir.dt.float32r
BF16 = mybir.dt.bfloat16
I32 = mybir.dt.int32
AF = mybir.ActivationFunctionType
OP = mybir.AluOpType
AX = mybir.AxisListType

N_CORES = 8
D = 1024
H = 4096
HS = 1024
E = 8
P = 128

TOK_BLOCK = 1024   # shared-expert token blocking
H_BLOCK_S = 1024   # shared-expert weight blocking
H_BLOCK_R = 512    # routed-expert weight blocking


def _chunks(n, step=512):
    out, o = [], 0
    while o < n:
        out.append((o, min(step, n - o)))
        o += step
    return out


def build_nc(n_tok: int, cap: int, num_devices: int = N_CORES):
    assert n_tok % TOK_BLOCK == 0 and cap % P == 0
    nc = bacc.Bacc("TRN2", target_bir_lowering=False, debug=False,
                   num_devices=num_devices)
    aps = {}

    def dram(name, shape, dt, kind="ExternalInput"):
        aps[name] = nc.dram_tensor(name, shape, dt, kind=kind).ap()

    TT = n_tok // P
    dram("xT", [D, n_tok], F32R)     # fp32, router only
    dram("xT16", [D, n_tok], BF16)   # bf16, FFN input
    dram("xrows", [n_tok, D], F32)
    dram("rn", [n_tok, E], F32)
    dram("wrn", [D, 2 * E], F32R)
    dram("brbnT", [2 * E, 1], F32)
    dram("esel", [P, E], F32)
    dram("ones32", [1, P], F32)
    dram("onescol", [P, 1], F32)
    dram("triu128", [P, P], F32)     # [j,i]=1 if j<i
    dram("triu32", [TT, TT], F32)
    dram("iotaf", [P, P], F32)       # [p,s] = s
    dram("ighl", [P, TT, 3], BF16)   # [:,tt,0]=tt, [:,tt,1]=p, [:,tt,2]=0
    dram("id128", [P, P], F32)
    dram("w1", [D, H], BF16)
    dram("b1", [P, H // P], F32)
    dram("w2", [H, D], BF16)
    dram("sw1", [D, HS], BF16)
    dram("sb1", [P, HS // P], F32)
    dram("sw2", [HS, D], BF16)
    dram("out_sh", [n_tok, D], BF16, kind="ExternalOutput")
    dram("out_rt", [cap, D], BF16, kind="ExternalOutput")
    dram("idx_t", [cap, 1], I32, kind="ExternalOutput")
    dram("gate_o", [cap, 1], F32, kind="ExternalOutput")
    dram("cnt_t", [1, 1], F32, kind="ExternalOutput")

    with tile.TileContext(nc) as tc:
        with ExitStack() as es:
            _emit(es, tc, nc, aps, n_tok, cap)
    nc.compile()
    return nc


def _emit(es, tc, nc, aps, n_tok, cap):
    TT = n_tok // P
    DS = D // P
    NTC = cap // P

    A = type("A", (), aps)

    cpool = es.enter_context(tc.tile_pool(name="const", bufs=1))
    rpool = es.enter_context(tc.tile_pool(name="router", bufs=2))
    spool = es.enter_context(tc.tile_pool(name="rscratch", bufs=1))
    gpool = es.enter_context(tc.tile_pool(name="gather", bufs=2))
    rpsum = es.enter_context(tc.tile_pool(name="rpsum", bufs=2, space="PSUM"))
    xpool = es.enter_context(tc.tile_pool(name="xb", bufs=2))
    w1pool = es.enter_context(tc.tile_pool(name="w1b", bufs=2))
    w2pool = es.enter_context(tc.tile_pool(name="w2b", bufs=1))
    hpool = es.enter_context(tc.tile_pool(name="hT", bufs=1))
    ypool = es.enter_context(tc.tile_pool(name="yacc", bufs=1))
    psum = es.enter_context(tc.tile_pool(name="psum", bufs=6, space="PSUM"))

    def ctile(shape, dt, name):
        return cpool.tile(shape, dt, name=name, tag=name)

    def stile(shape, name, dt=F32, bufs=None):
        return spool.tile(shape, dt, name=name, tag=name, bufs=bufs)

    def rps(shape, name):
        return rpsum.tile(shape, F32, name=name, tag="rps")

    def load_const(name, shape, dt):
        t = ctile(shape, dt, name + "_sb")
        nc.sync.dma_start(t[:], aps[name][:])
        return t

    # ---- constants ----
    wrn_sb = ctile([P, DS, 2 * E], F32R, "wrn_sb")
    nc.sync.dma_start(wrn_sb[:], A.wrn.rearrange("(ds p) e -> p ds e", p=P))
    brbnT_sb = load_const("brbnT", [2 * E, 1], F32)
    esel_sb = load_const("esel", [P, E], F32)
    ones32_sb = load_const("ones32", [1, P], F32)
    onescol_sb = load_const("onescol", [P, 1], F32)
    triu128_sb = load_const("triu128", [P, P], F32)
    triu32_sb = load_const("triu32", [TT, TT], F32)
    iotaf_sb = load_const("iotaf", [P, P], F32)
    id_sb = load_const("id128", [P, P], F32)
    b1_sb = load_const("b1", [P, H // P], F32)
    sb1_sb = load_const("sb1", [P, HS // P], F32)

    # ---- FFN helpers (fp32r) ----
    # Chunk-inner loops keep the stationary operand loaded across all
    # output chunks (PE pulls LDWEIGHTS ahead only between weight changes),
    # accumulating into several PSUM banks concurrently.
    def gemm1(xsrc, nb, w1b_t, hT_t, bias_sb, bias_off, nsub):
        ch = _chunks(nb)
        for hs in range(nsub):
            pss = [psum.tile([P, 512], F32, name="ps_g1", tag="ps")
                   for _ in ch]
            for ds in range(DS):
                for ci, (no, nw) in enumerate(ch):
                    nc.tensor.matmul(
                        pss[ci][:, :nw], w1b_t[:, ds, hs * P:(hs + 1) * P],
                        xsrc[:, ds, no:no + nw],
                        start=(ds == 0), stop=(ds == DS - 1))
            for ci, (no, nw) in enumerate(ch):
                nc.scalar.activation(
                    hT_t[:, hs, no:no + nw], pss[ci][:, :nw], AF.Relu,
                    bias=bias_sb[:, bias_off + hs:bias_off + hs + 1])

    def gemm2(y_acc, hT_t, w2b_t, nt, nsub, first, tt_done=None):
        ch = _chunks(D)
        for tt in range(nt):
            pss = [psum.tile([P, 512], F32, name="ps_g2", tag="ps")
                   for _ in ch]
            for hs in range(nsub):
                for ci, (do, dw) in enumerate(ch):
                    nc.tensor.matmul(
                        pss[ci][:, :dw], hT_t[:, hs, tt * P:(tt + 1) * P],
                        w2b_t[:, hs, do:do + dw],
                        start=(hs == 0), stop=(hs == nsub - 1))
            for ci, (do, dw) in enumerate(ch):
                ys = y_acc[:, tt, do:do + dw]
                if first:
                    nc.scalar.activation(ys, pss[ci][:, :dw], AF.Copy)
                else:
                    nc.vector.tensor_add(ys, ys, pss[ci][:, :dw])
            if tt_done is not None:
                tt_done(tt)

    # ---- router phase (fp32r, weights-stationary) ----
    # wrn [d,16] is the stationary operand (16-column LDWEIGHTS is ~free);
    # x streams as the moving operand in 256-token chunks, accumulating
    # over ds into a [16, 256] PSUM; PE-transpose restores [tok, 16].
    # The fp32 x chunks of the first two token blocks are also cast to
    # bf16 on-chip (DVE) so shared blocks 0/1 need no separate x DMA.
    RCH = 256
    NB = TOK_BLOCK
    lgnl = stile([P, TT, 2 * E], "lgnl")
    xb16 = {}
    for rc in range(n_tok // RCH):
        xt_r = rpool.tile([P, DS, RCH], F32R, name="xt_r")
        eng = nc.sync if rc % 2 == 0 else nc.gpsimd
        eng.dma_start(
            xt_r[:],
            A.xT[:, rc * RCH:(rc + 1) * RCH].rearrange(
                "(ds p) t -> p ds t", p=P))
        ps = rps([2 * E, RCH], "ps_r")
        for ds in range(DS):
            nc.tensor.matmul(ps[:], wrn_sb[:, ds, :], xt_r[:, ds, :],
                             start=(ds == 0), stop=(ds == DS - 1))
        if rc < 2 * (NB // RCH):
            b, off = rc // (NB // RCH), (rc % (NB // RCH)) * RCH
            if off == 0:
                xb16[b] = xpool.tile([P, DS, NB], BF16, name="xb", tag="xb")
            nc.vector.tensor_copy(xb16[b][:, :, off:off + RCH],
                                  xt_r[:].bitcast(F32))
        lgT = stile([2 * E, RCH], "lgT", bufs=2)
        nc.vector.tensor_scalar(lgT[:], ps[:], brbnT_sb[:], None, op0=OP.add)
        for q in range(RCH // P):
            tt = (rc * RCH) // P + q
            tps2 = rpsum.tile([P, 2 * E], F32, name="tps2", tag="rps")
            nc.tensor.transpose(tps2[:], lgT[:, q * P:(q + 1) * P],
                                id_sb[:2 * E, :2 * E])
            nc.scalar.activation(lgnl[:, tt, :], tps2[:], AF.Copy)

    gate = stile([P, TT], "gate")
    mask = stile([P, TT], "mask")
    RC = 8
    for c0 in range(0, TT, RC):
        lg = lgnl[:, c0:c0 + RC, 0:E]
        nl = lgnl[:, c0:c0 + RC, E:2 * E]
        shp = [P, RC, E]

        e0 = stile(shp, "e0"); nc.scalar.activation(e0[:], nl, AF.Exp)
        l0 = stile(shp, "l0"); nc.scalar.activation(l0[:], e0[:], AF.Ln)
        r0 = stile(shp, "r0"); nc.vector.tensor_sub(r0[:], nl, l0[:])
        t0 = stile(shp, "t0"); nc.vector.tensor_mul(t0[:], e0[:], r0[:])
        ee = stile(shp, "ee"); nc.vector.tensor_add(ee[:], e0[:], t0[:])
        uu = stile(shp, "uu"); nc.vector.tensor_scalar_add(uu[:], ee[:], 1.0)
        s0 = stile(shp, "s0"); nc.scalar.activation(s0[:], uu[:], AF.Ln)
        e1 = stile(shp, "e1"); nc.scalar.activation(e1[:], s0[:], AF.Exp)
        l1 = stile(shp, "l1"); nc.scalar.activation(l1[:], e1[:], AF.Ln)
        r1 = stile(shp, "r1"); nc.vector.tensor_sub(r1[:], s0[:], l1[:])
        t1 = stile(shp, "t1"); nc.vector.tensor_mul(t1[:], e1[:], r1[:])
        e1p = stile(shp, "e1p"); nc.vector.tensor_add(e1p[:], e1[:], t1[:])
        re1 = stile(shp, "re1"); nc.vector.reciprocal(re1[:], e1p[:])
        dd = stile(shp, "dd"); nc.vector.tensor_mul(dd[:], uu[:], re1[:])
        dm = stile(shp, "dm"); nc.vector.tensor_scalar_add(dm[:], dd[:], -1.0)
        sp = stile(shp, "sp"); nc.vector.tensor_add(sp[:], s0[:], dm[:])

        rn_sb = stile(shp, "rn_sb")
        nc.gpsimd.dma_start(
            rn_sb[:],
            A.rn[c0 * P:(c0 + RC) * P, :].rearrange("(t p) e -> p t e", p=P))
        noise = stile(shp, "noise"); nc.vector.tensor_mul(noise[:], rn_sb[:], sp[:])
        noisy = stile(shp, "noisy"); nc.vector.tensor_add(noisy[:], lg, noise[:])

        m1 = stile([P, RC], "m1")
        nc.vector.tensor_reduce(m1[:], noisy[:], axis=AX.X, op=OP.max)
        m1b = m1[:, :, None].broadcast_to(shp)
        eq = stile(shp, "eq")
        nc.vector.tensor_tensor(eq[:], noisy[:], m1b, op=OP.is_equal)
        big = stile(shp, "big"); nc.vector.tensor_scalar_mul(big[:], eq[:], 1e30)
        noisy2 = stile(shp, "noisy2"); nc.vector.tensor_sub(noisy2[:], noisy[:], big[:])
        m2 = stile([P, RC], "m2")
        nc.vector.tensor_reduce(m2[:], noisy2[:], axis=AX.X, op=OP.max)
        m2b = m2[:, :, None].broadcast_to(shp)
        ge = stile(shp, "ge")
        nc.vector.tensor_tensor(ge[:], noisy[:], m2b, op=OP.is_ge)
        shd = stile(shp, "shd"); nc.vector.tensor_sub(shd[:], noisy[:], m1b)
        ex = stile(shp, "ex"); nc.scalar.activation(ex[:], shd[:], AF.Exp)
        gg = stile(shp, "gg"); nc.vector.tensor_mul(gg[:], ex[:], ge[:])
        den = stile([P, RC], "den")
        nc.vector.tensor_reduce(den[:], gg[:], axis=AX.X, op=OP.add)
        rden = stile([P, RC], "rden")
        nc.vector.reciprocal(rden[:], den[:])
        gate8 = stile(shp, "gate8")
        nc.vector.tensor_tensor(gate8[:], gg[:],
                                rden[:, :, None].broadcast_to(shp), op=OP.mult)
        gsel = stile(shp, "gsel")
        nc.vector.tensor_tensor(gsel[:], gate8[:],
                                esel_sb[:, None, :].broadcast_to(shp), op=OP.mult)
        nc.vector.tensor_reduce(gate[:, c0:c0 + RC], gsel[:], axis=AX.X, op=OP.add)
        msel = stile(shp, "msel")
        nc.vector.tensor_tensor(msel[:], ge[:],
                                esel_sb[:, None, :].broadcast_to(shp), op=OP.mult)
        nc.vector.tensor_reduce(mask[:, c0:c0 + RC], msel[:], axis=AX.X, op=OP.add)

    # ---- shared experts ----
    NT = NB // P

    def shared_block(b):
        tok0 = b * NB
        if b in xb16:
            xb = xb16[b]            # cast on-chip during the router phase
        else:
            xb = xpool.tile([P, DS, NB], BF16, name="xb", tag="xb")
            nc.sync.dma_start(
                xb[:],
                A.xT16[:, tok0:tok0 + NB].rearrange("(ds p) t -> p ds t", p=P))
        y_s = ypool.tile([P, NT, D], BF16, name="y_s", tag="y_acc")
        HSUB_S = H_BLOCK_S // P
        for hb in range(HS // H_BLOCK_S):
            sw1b = w1pool.tile([P, DS, H_BLOCK_S], BF16, name="sw1b", tag="w1b")
            for (ho_, hw_) in (_chunks(H_BLOCK_S, 128) if b == 0 else [(0, H_BLOCK_S)]):
                nc.sync.dma_start(
                    sw1b[:, :, ho_:ho_ + hw_],
                    A.sw1[:, hb * H_BLOCK_S + ho_:
                          hb * H_BLOCK_S + ho_ + hw_].rearrange(
                        "(ds p) h -> p ds h", p=P))
            hTs = hpool.tile([P, HSUB_S, NB], BF16, name="hTs", tag="hTb")
            gemm1(xb, NB, sw1b, hTs, sb1_sb, hb * HSUB_S, HSUB_S)
            sw2b = w2pool.tile([P, HSUB_S, D], BF16, name="sw2b", tag="w2b")
            nc.sync.dma_start(
                sw2b[:], A.sw2[hb * H_BLOCK_S:(hb + 1) * H_BLOCK_S, :].rearrange(
                    "(hs p) d -> p hs d", p=P))
            gemm2(y_s, hTs, sw2b, NT, HSUB_S, first=(hb == 0))
        for tt in range(NT):
            nc.sync.dma_start(A.out_sh[tok0 + tt * P:tok0 + (tt + 1) * P, :],
                              y_s[:, tt, :])

    # Shared block 0 runs while the router/gate pipeline drains; compaction
    # + slot tables (+ per-slot-tile gather launches) are emitted BETWEEN
    # shared blocks so the gather DMA and small PE ops overlap shared FFN
    # instead of stalling the routed FFN at the end.
    shared_block(0)

    # ---- compaction: slot = prefix(mask); unselected -> `cap` (no slot) --
    cntp = rps([TT, 1], "cntp")
    nc.tensor.matmul(cntp[:], mask[:], onescol_sb[:], start=True, stop=True)
    cnt_sb = stile([TT, 1], "cnt_sb")
    nc.scalar.activation(cnt_sb[:], cntp[:], AF.Copy)
    ecsp = rps([1, TT], "ecsp")
    nc.tensor.matmul(ecsp[:], cnt_sb[:], triu32_sb[:], start=True, stop=True)
    ecs_row = stile([1, TT], "ecs_row")
    nc.scalar.activation(ecs_row[:], ecsp[:], AF.Copy)
    totp = rps([1, 1], "totp")
    nc.tensor.matmul(totp[:], cnt_sb[:], onescol_sb[:TT, :], start=True, stop=True)
    tot_sb = stile([1, 1], "tot_sb")
    nc.scalar.activation(tot_sb[:], totp[:], AF.Copy)
    nc.sync.dma_start(A.cnt_t[:], tot_sb[:])

    posp = rps([P, TT], "posp")
    nc.tensor.matmul(posp[:], triu128_sb[:], mask[:], start=True, stop=False)
    nc.tensor.matmul(posp[:], ones32_sb[:1, :], ecs_row[:1, :],
                     start=False, stop=True)
    pos = stile([P, TT], "pos")
    nc.scalar.activation(pos[:], posp[:], AF.Copy)
    # pos_final = pos*mask + (1-mask)*cap
    pm_a = stile([P, TT], "pm_a"); nc.vector.tensor_mul(pm_a[:], pos[:], mask[:])
    pm_b = stile([P, TT], "pm_b")
    nc.vector.tensor_scalar_mul(pm_b[:], mask[:], float(cap))
    pm_c = stile([P, TT], "pm_c"); nc.vector.tensor_sub(pm_c[:], pm_a[:], pm_b[:])
    pm = stile([P, TT], "pm")
    nc.vector.tensor_scalar_add(pm[:], pm_c[:], float(cap))

    # ---- slot tables via one-hot permutation matmuls (bf16, exact:
    # values are 0/1 and small ints; accumulation is fp32 PSUM).
    # igr [tok,3] is the STATIONARY operand (3-column LDWEIGHTS is ~free),
    # the one-hot tile streams; the [3,128] result is PE-transposed back.
    # Gathers launch per slot-tile so they overlap shared block 1. ----
    igr = stile([P, TT, 3], "igr", BF16)
    nc.sync.dma_start(igr[:], A.ighl[:])
    nc.vector.tensor_copy(igr[:, :, 2], gate[:])
    ig_sb = stile([P, NTC, 3], "ig_sb")
    idxf = stile([P, NTC], "idxf")
    idx_g = stile([P, NTC], "idx_g", I32)
    gate_g = stile([P, NTC], "gate_g")
    xgs = []
    for st in range(NTC):
        ps_ig = rps([3, P], "ps_ig")
        for tt in range(TT):
            pshift = stile([P, 1], "pshift", bufs=3)
            nc.vector.tensor_scalar_add(pshift[:], pm[:, tt:tt + 1],
                                        -float(st * P))
            ptile = stile([P, P], "ptile", BF16, bufs=2)
            nc.vector.tensor_scalar(ptile[:], iotaf_sb[:], pshift[:], None,
                                    op0=OP.is_equal)
            nc.tensor.matmul(ps_ig[:], igr[:, tt, :], ptile[:],
                             start=(tt == 0), stop=(tt == TT - 1))
        igT = stile([3, P], "igT", bufs=2)
        nc.scalar.activation(igT[:], ps_ig[:], AF.Copy)
        tpsi = rpsum.tile([P, 3], F32, name="tpsi", tag="rps")
        nc.tensor.transpose(tpsi[:], igT[:], id_sb[:3, :3])
        nc.scalar.activation(ig_sb[:, st, :], tpsi[:], AF.Copy)
        # idx = hi*128 + lo ; gate_g = col 2 ; launch this tile's gather
        nc.vector.tensor_scalar(idxf[:, st:st + 1], ig_sb[:, st, 0:1],
                                float(P), None, op0=OP.mult)
        nc.vector.tensor_add(idxf[:, st:st + 1], idxf[:, st:st + 1],
                             ig_sb[:, st, 1:2])
        nc.vector.tensor_copy(idx_g[:, st:st + 1], idxf[:, st:st + 1])
        nc.vector.tensor_copy(gate_g[:, st:st + 1], ig_sb[:, st, 2:3])
        xg = gpool.tile([P, D], F32, name="xg", tag="xg", bufs=4)
        nc.gpsimd.indirect_dma_start(
            out=xg[:], in_=A.xrows[:],
            in_offset=IndirectOffsetOnAxis(ap=idx_g[:, st:st + 1], axis=0),
            out_offset=None)
        xgs.append(xg)
    nc.sync.dma_start(A.idx_t.rearrange("(st p) o -> p (st o)", p=P), idx_g[:])
    nc.sync.dma_start(A.gate_o.rearrange("(st p) o -> p (st o)", p=P), gate_g[:])

    shared_block(1)

    # ---- transpose gathered x rows to xgT [d, slot] ----
    xgT = xpool.tile([P, DS, cap], BF16, name="xgT", tag="xgT", bufs=1)
    for st in range(NTC):
        for dp in range(DS):
            tps = rps([P, P], "tps")
            nc.tensor.transpose(tps[:], xgs[st][:, dp * P:(dp + 1) * P],
                                id_sb[:])
            nc.scalar.activation(xgT[:, dp, st * P:(st + 1) * P], tps[:], AF.Copy)

    # remaining shared blocks overlap the gather/table tail
    shared_block(2)
    shared_block(3)

    # ---- routed FFN on gathered tokens ----
    y_acc = ypool.tile([P, NTC, D], F32, name="y_acc", tag="y_acc")
    HSUB_R = H_BLOCK_R // P
    NHB = H // H_BLOCK_R

    def emit_gating(tt):
        yg16 = gpool.tile([P, D], BF16, name="yg16", tag="yg16")
        nc.vector.tensor_scalar(yg16[:], y_acc[:, tt, :],
                                gate_g[:, tt:tt + 1], None, op0=OP.mult)
        nc.sync.dma_start(A.out_rt[tt * P:(tt + 1) * P, :], yg16[:])

    for hb in range(NHB):
        w1b = w1pool.tile([P, DS, H_BLOCK_R], BF16, name="w1b", tag="w1b")
        for hs_ in range(H_BLOCK_R // P):
            nc.sync.dma_start(
                w1b[:, :, hs_ * P:(hs_ + 1) * P],
                A.w1[:, hb * H_BLOCK_R + hs_ * P:
                     hb * H_BLOCK_R + (hs_ + 1) * P].rearrange(
                    "(ds p) h -> p ds h", p=P))
        hTb = hpool.tile([P, HSUB_R, cap], BF16, name="hTb", tag="hTb")
        gemm1(xgT, cap, w1b, hTb, b1_sb, hb * HSUB_R, HSUB_R)
        w2b = w2pool.tile([P, HSUB_R, D], BF16, name="w2b", tag="w2b")
        nc.sync.dma_start(
            w2b[:], A.w2[hb * H_BLOCK_R:(hb + 1) * H_BLOCK_R, :].rearrange(
                "(hs p) d -> p hs d", p=P))
        gemm2(y_acc, hTb, w2b, NTC, HSUB_R, first=(hb == 0),
              tt_done=emit_gating if hb == NHB - 1 else None)


# ---------------- host side ----------------

_NC_CACHE = {}
CAP = 1152


def _get_nc(n_tok, cap):
    key = (n_tok, cap)
    if key not in _NC_CACHE:
        _NC_CACHE[key] = build_nc(n_tok, cap)
    return _NC_CACHE[key]


def make_in_maps(n_tok, cap, x, router_noise, Wr, br, Wn, bn, rW1, rb1, rW2,
                 rb2, sW1, sb1, sW2, sb2):
    import ml_dtypes
    BF = ml_dtypes.bfloat16
    TT = n_tok // P
    xf = np.ascontiguousarray(x.reshape(n_tok, D))
    xT = np.ascontiguousarray(xf.T)
    xT16 = xT.astype(BF)
    rnf = np.ascontiguousarray(router_noise.reshape(n_tok, E)).astype(np.float32)
    wrn = np.ascontiguousarray(np.concatenate([Wr, Wn], axis=1)).astype(np.float32)
    brbnT = np.concatenate([br, bn]).reshape(2 * E, 1).astype(np.float32)
    ones = np.ones((1, P), np.float32)
    ighl = np.zeros((P, TT, 3), np.float32)
    ighl[:, :, 0] = np.arange(TT)[None, :]
    ighl[:, :, 1] = np.arange(P)[:, None]
    ighl = ighl.astype(BF)

    in_maps = []
    for c in range(N_CORES):
        se, hsl = c // 4, (c % 4) * HS
        esel = np.zeros((P, E), np.float32)
        esel[:, c] = 1.0
        in_maps.append({
            "xT": xT,
            "xT16": xT16,
            "xrows": xf,
            "rn": rnf,
            "wrn": wrn,
            "brbnT": brbnT,
            "esel": esel,
            "ones32": ones,
            "onescol": np.ones((P, 1), np.float32),
            "triu128": np.triu(np.ones((P, P), np.float32), 1),
            "triu32": np.triu(np.ones((TT, TT), np.float32), 1),
            "iotaf": np.tile(np.arange(P, dtype=np.float32)[None, :], (P, 1)),
            "ighl": ighl,
            "id128": np.eye(P, dtype=np.float32),
            "w1": np.ascontiguousarray(rW1[c]).astype(BF),
            "b1": np.ascontiguousarray(rb1[c].reshape(H // P, P).T),
            "w2": np.ascontiguousarray(rW2[c]).astype(BF),
            "sw1": np.ascontiguousarray(sW1[se][:, hsl:hsl + HS]).astype(BF),
            "sb1": np.ascontiguousarray(
                sb1[se][hsl:hsl + HS].reshape(HS // P, P).T),
            "sw2": np.ascontiguousarray(sW2[se][hsl:hsl + HS, :]).astype(BF),
        })
    return in_maps


def combine(x, results, n_tok, cap, rb2, sb2):
    acc = x.reshape(n_tok, D).astype(np.float32).copy()
    acc += sb2.sum(axis=0).astype(np.float32)
    for c in range(N_CORES):
        acc += results[c]["out_sh"].astype(np.float32)
    for c in range(N_CORES):
        n = int(round(float(results[c]["cnt_t"][0, 0])))
        assert n <= cap, f"core {c}: count {n} exceeds capacity {cap}"
        idx = results[c]["idx_t"][:n, 0]
        g = results[c]["gate_o"][:n]
        acc[idx] += results[c]["out_rt"][:n].astype(np.float32) + g * rb2[c][None, :]
    return acc


def kernel(x, router_noise, topk, Wr, br, Wn, bn, rW1, rb1, rW2, rb2,
           sW1, sb1, sW2, sb2, _trace=False):
    assert int(topk) == 2
    x = np.asarray(x, np.float32)
    B, T, Dx = x.shape
    n_tok = B * T
    nc = _get_nc(n_tok, CAP)
    in_maps = make_in_maps(
        n_tok, CAP, x, np.asarray(router_noise, np.float32),
        np.asarray(Wr, np.float32), np.asarray(br, np.float32),
        np.asarray(Wn, np.float32), np.asarray(bn, np.float32),
        np.asarray(rW1, np.float32), np.asarray(rb1, np.float32),
        np.asarray(rW2, np.float32), np.asarray(rb2, np.float32),
        np.asarray(sW1, np.float32), np.asarray(sb1, np.float32),
        np.asarray(sW2, np.float32), np.asarray(sb2, np.float32))
    res = run_bass_kernel_spmd(nc, in_maps, core_ids=list(range(N_CORES)),
                               trace=_trace)
    out = combine(x, res.results, n_tok, CAP,
                  np.asarray(rb2, np.float32),
                  np.asarray(sb2, np.float32)).reshape(B, T, Dx)
    if _trace:
        return out, res
    return out

